# revision 24
# baseline (speedup 1.0000x reference)
"""LookupConv2d Trainium2 kernel — Winograd F(2,3) along H.

Math: out = conv2d(x, W), W[o] = sum_s coeff[o,s] * dictionary[idx[o,s]].
Factorization: out = M @ conv2d(x, dictionary) with M the (512, 100) scatter
of the coefficients — a 100-channel conv followed by a 1x1 512x100 mix.

The 3x3 conv is computed with 1D Winograd F(2,3) applied along H (the
strided spatial dim) and direct taps along W (the contiguous dim):
    U[c, b, th, w]   = sum_i BT[b, i] x_p[c, 2*th + i, w]    (DVE/Pool +-1 ops)
    Yw[d, b, th, w]  = sum_{c, tj} Wt[d, c, b, tj] U[c, b, th, w+tj]  (PE)
    y[d, 2*th+e, w]  = sum_b AT[e, b] Yw[d, b, th, w]        (DVE, from PSUM)
with Wt = G-transformed dictionary taps (host-side).  This streams
4 positions x 3 w-taps = 12 PE rows per 2 output rows instead of direct
conv's 18 per 2 — 1.5x less TensorE work.

Precision: bf16 matmul operands, f32 PSUM accumulation, f32 output
transform; rel err ~5e-3 (gate 2e-2).

Sharding: data-parallel over batch N=16 -> 2 images per core on 8 cores.

Structure per image: 7 pairs of 8 output rows (4 winograd tiles).  The 48
conv matmuls of a pair accumulate into ONE 2-bank PSUM tile laid out
[100, half, b, th, 64] (w padded to 64 so each (half, b) region is 512B and
never crosses a bank).  Back half per pair: DVE output transform straight
from PSUM (f32, no intermediate copy), 4 mix matmuls, og copies on ACT, one
8-row output DMA (896B descriptors).  Input transform ops split DVE/Pool.
PE pipeline: conv(pair i) runs while the back half of pair i-1 drains; the
last pair is processed in 4-row halves to shorten the tail.  Warmup
matmuls burn the PE p-state ramp while the first DMAs land; the w matrix is
DMA'd in b-major pieces so the first conv matmuls can start early.
"""

import numpy as np

N_CORES = 8
IMGS_PER_CORE = 2
CIN = 256
COUT = 512
NDICT = 100
H = W = 56
WPAD = 64  # psum row padded to 64 f32 so (half, b) regions are bank-aligned
HP = WP = 58  # padded input
NTH = 28           # winograd tiles along H
N_PAIRS = 7        # 4 tiles / 8 output rows each
NWARM = 128

TRACE = False  # set by test.py to get a profile
_LAST_RESULTS = {}  # test.py reads exec_time_ns from here

# F(2,3): BT = [[1,0,-1,0],[0,1,1,0],[0,-1,1,0],[0,1,0,-1]]
#         AT = [[1,1,1,0],[0,1,-1,-1]]
G2 = np.array([[1, 0, 0], [0.5, 0.5, 0.5], [0.5, -0.5, 0.5], [0, 0, 1]],
              dtype=np.float32)


def _build_program():
    import concourse.bacc as bacc
    import concourse.mybir as mybir
    import concourse.tile as tile

    f32 = mybir.dt.float32
    bf16 = mybir.dt.bfloat16
    ADD = mybir.AluOpType.add
    SUB = mybir.AluOpType.subtract

    nc = bacc.Bacc("TRN2", target_bir_lowering=False, debug=False)

    x_d = nc.dram_tensor("x", (IMGS_PER_CORE, 2, 128, HP, WP), bf16,
                         kind="ExternalInput")
    # w layout: [c, (b, cb, tj, d)] — b-major so it can stream in b pieces
    w_d = nc.dram_tensor("w", (128, 4 * 2 * 3 * NDICT), bf16,
                         kind="ExternalInput")
    mt_d = nc.dram_tensor("mt", (NDICT, COUT), bf16, kind="ExternalInput")
    out_d = nc.dram_tensor("out", (IMGS_PER_CORE, 4, 128, H, WPAD), bf16,
                           kind="ExternalOutput")

    WB = 2 * 3 * NDICT  # 600 cols per b piece

    with tile.TileContext(nc) as tc:
        with (
            tc.tile_pool(name="consts", bufs=1) as consts,
            tc.tile_pool(name="xpool", bufs=2) as xpool,
            tc.tile_pool(name="upool", bufs=3) as upool,
            tc.tile_pool(name="scrpool", bufs=2) as scrpool,
            tc.tile_pool(name="ypool", bufs=2) as ypool,
            tc.tile_pool(name="ogpool", bufs=4) as ogpool,
            tc.tile_pool(name="psum_y", bufs=2, space="PSUM") as psum_y,
            tc.tile_pool(name="psum_o", bufs=4, space="PSUM") as psum_o,
        ):
            # PE p-state warmup: tiny dependency-free matmuls keep the PE
            # busy through the 0.65->1.2->2.4 GHz ramp while the first input
            # DMAs land.
            warm_sb = consts.tile([128, 32], bf16, name="warm_sb")
            nc.vector.memset(warm_sb[:], 0.0)
            pwarm = psum_y.tile([NDICT, 2, 4, 2, WPAD], f32, tag="py",
                                name="pwarm")
            for _ in range(NWARM):
                nc.tensor.matmul(pwarm[:8, 0, 0, 0, :32], warm_sb[:, 0:8],
                                 warm_sb[:], start=True, stop=True)

            w_sb = consts.tile([128, 4 * WB], bf16, name="w_sb")
            mt_sb = consts.tile([NDICT, COUT], bf16, name="mt_sb")

            x_v = x_d.rearrange("i b c h w -> c i b h w")
            out_v = out_d.rearrange("i o c h w -> c i o h w")

            # DMA order tuned for the pipeline head: first x rows, then the
            # w pieces in b order (the first conv matmuls need b=0 only),
            # then the rest of the input.
            x_tiles = {i: xpool.tile([128, 2, HP, WP], bf16, tag="xt",
                                     name=f"xt{i}")
                       for i in range(2)}

            def dma_x(img, r0, nr):
                nc.sync.dma_start(x_tiles[img][:, :, r0:r0 + nr, :],
                                  x_v[:, img, :, r0:r0 + nr, :])

            def dma_w(b):
                nc.sync.dma_start(w_sb[:, b * WB:(b + 1) * WB],
                                  w_d[:, b * WB:(b + 1) * WB])

            dma_x(0, 0, 10)
            dma_w(0)
            dma_x(0, 10, 9)
            dma_w(1)
            dma_w(2)
            dma_w(3)
            dma_x(0, 19, 15)
            nc.sync.dma_start(mt_sb[:], mt_d[:])
            dma_x(0, 34, 24)
            dma_x(1, 0, 20)
            dma_x(1, 20, 19)
            dma_x(1, 39, 19)

            def emit_u(img, p):
                """input transform for a pair: 4 ops, split DVE/Pool."""
                th0 = 4 * p
                xv = x_tiles[img][:].rearrange("c b (t q) w -> c b t q w",
                                               q=2)
                ut = upool.tile([128, 2, 4, 4, WP], bf16, tag="ut")
                r0e = xv[:, :, th0:th0 + 4, 0, :]       # rows 2*th
                r1o = xv[:, :, th0:th0 + 4, 1, :]       # rows 2*th+1
                r2e = xv[:, :, th0 + 1:th0 + 5, 0, :]   # rows 2*th+2
                r3o = xv[:, :, th0 + 1:th0 + 5, 1, :]   # rows 2*th+3
                # first pair of an image feeds a draining pipeline: keep all
                # ops on the fast engine so conv isn't gated by Pool latency
                eng0 = (nc.vector if (img == 0 and p <= 2) or p == 0
                        or p == N_PAIRS - 1 else nc.gpsimd)
                nc.vector.tensor_tensor(ut[:, :, 0, :, :], r0e, r2e, SUB)
                nc.vector.tensor_tensor(ut[:, :, 1, :, :], r1o, r2e, ADD)
                eng0.tensor_tensor(ut[:, :, 2, :, :], r2e, r1o, SUB)
                eng0.tensor_tensor(ut[:, :, 3, :, :], r1o, r3o, SUB)
                return ut

            def emit_conv(ut, halves=(0, 1)):
                """conv matmuls into one pair-spanning 2-bank PSUM tile."""
                py = psum_y.tile([NDICT, 2, 4, 2, WPAD], f32, tag="py")
                for b in range(4):
                    for half in halves:
                        k = 0
                        for cb in range(2):
                            for tj in range(3):
                                s = (b * 2 + cb) * 3 * NDICT + tj * NDICT
                                nc.tensor.matmul(
                                    py[:, half, b, :, 0:W],
                                    w_sb[:, s:s + NDICT],
                                    ut[:, cb, b, 2 * half:2 * half + 2,
                                       tj:tj + W],
                                    start=(k == 0), stop=(k == 5))
                                k += 1
                return py

            def emit_outt(py, halves=(0, 1)):
                """output transform (DVE, from PSUM).

                HW allows only one PSUM input per op, so use b-axis
                reductions (single input):
                    e0 = m0 + m1 + m2 = reduce(b 0..2)
                    e1 = m1 - (m2 + m3) = m1 - reduce(b 2..3)
                """
                hs = slice(halves[0], halves[-1] + 1)
                pv = py[:].rearrange("d h b t w -> d h t w b")
                scr = scrpool.tile([NDICT, 2, 2, W], f32, tag="scr")
                # y: [100, half, th, e, w] -> rows h = half*4 + th*2 + e
                y = ypool.tile([NDICT, 2, 2, 2, W], bf16, tag="y")
                sv = scr[:, hs, :, :]
                yv = y[:, hs, :, :, :]
                with nc.allow_low_precision(
                        reason="3-term reduce, f32 in, single rounding"):
                    nc.vector.tensor_reduce(
                        yv[:, :, :, 0, :], pv[:, hs, :, 0:W, 0:3],
                        mybir.AxisListType.X, ADD)
                    nc.vector.tensor_reduce(
                        sv, pv[:, hs, :, 0:W, 2:4],
                        mybir.AxisListType.X, ADD)
                nc.vector.tensor_tensor(yv[:, :, :, 1, :],
                                        py[:, hs, 1, :, 0:W], sv, SUB)
                return y

            def emit_back(y, img, p, halves=(0, 1)):
                """mix, og copies, out DMA."""
                hs = slice(halves[0], halves[-1] + 1)
                nh = len(halves)
                yv = y[:, hs, :, :, :]

                og = ogpool.tile([128, 4, nh * 4, WPAD], bf16, tag="og")
                r0 = 8 * p + 4 * halves[0]
                taper = nh == 1
                final = taper and halves[0] == 1
                for ob in range(4):
                    po = psum_o.tile([128, nh * 4 * W], f32, tag="po")
                    nc.tensor.matmul(po[:], mt_sb[:, ob * 128:(ob + 1) * 128],
                                     yv, start=True, stop=True)
                    pv = po[:].rearrange("c (h w) -> c h w", w=W)
                    if final and ob % 2 == 1:
                        # last rows: split og across ACT/DVE (nothing left
                        # for DVE to transform, shortens the drain)
                        nc.vector.tensor_copy(og[:, ob, :, 0:W], pv)
                    else:
                        nc.scalar.copy(og[:, ob, :, 0:W], pv)
                    if taper and ob % 2 == 1:
                        nc.sync.dma_start(
                            out_v[:, img, ob - 1:ob + 1, r0:r0 + nh * 4, :],
                            og[:, ob - 1:ob + 1, :, :])
                if not taper:
                    nc.sync.dma_start(out_v[:, img, :, r0:r0 + nh * 4, :],
                                      og[:])

            # software-pipeline by one pair: PE runs pair i's conv while the
            # other engines drain pair i-1.  The final pair is split into
            # two 4-row halves to shorten the post-conv tail.
            pending = None
            for img in range(IMGS_PER_CORE):
                for p in range(N_PAIRS):
                    last = img == IMGS_PER_CORE - 1 and p == N_PAIRS - 1
                    ut = emit_u(img, p)
                    if not last:
                        py = emit_conv(ut)
                        if pending is not None:
                            yp, pimg, pp = pending
                            emit_back(emit_outt(yp), pimg, pp)
                        pending = (py, img, p)
                    else:
                        py = emit_conv(ut, halves=(0,))
                        yp, pimg, pp = pending
                        emit_back(emit_outt(yp), pimg, pp)
                        ya = emit_outt(py, halves=(0,))
                        py2 = emit_conv(ut, halves=(1,))
                        emit_back(ya, img, p, halves=(0,))
                        yb = emit_outt(py2, halves=(1,))
                        emit_back(yb, img, p, halves=(1,))
    nc.compile()
    return nc


_NC_CACHE = None


def kernel(x, dictionary, lookup_indices, lookup_coefficients):
    global _NC_CACHE
    import ml_dtypes
    from concourse import bass_utils

    bf16 = ml_dtypes.bfloat16

    x = np.asarray(x, dtype=np.float32)
    dictionary = np.asarray(dictionary, dtype=np.float32)
    idx = np.asarray(lookup_indices).astype(np.int64)
    coef = np.asarray(lookup_coefficients, dtype=np.float32)

    # M^T[d, o] = sum_s coeff[o, s] * [idx[o, s] == d]
    mt = np.zeros((NDICT, COUT), np.float32)
    np.add.at(mt, (idx.reshape(-1),
                   np.repeat(np.arange(COUT), 3)), coef.reshape(-1))

    # winograd-transformed taps, b-major:
    # wt[c, (b, cb, tj, d)] = sum_i G2[b, i] dict[d, cb*128+c, i, tj]
    dte = dictionary.reshape(NDICT, 2, 128, 3, 3)
    wt = np.einsum("bi,dzcij->cbzjd", G2, dte)
    wt = np.ascontiguousarray(wt).reshape(128, 4 * 2 * 3 * NDICT)

    xp = np.pad(x, ((0, 0), (0, 0), (1, 1), (1, 1)))
    # [core, img, cblk, cin_in_block, hp, wp]
    xp = np.ascontiguousarray(
        xp.reshape(N_CORES, IMGS_PER_CORE, 2, 128, HP, WP))

    xb = xp.astype(bf16)
    wb = wt.astype(bf16)
    mb = mt.astype(bf16)

    if _NC_CACHE is None:
        _NC_CACHE = _build_program()
    nc = _NC_CACHE

    in_maps = [{"x": xb[i], "w": wb, "mt": mb} for i in range(N_CORES)]
    try:
        res = bass_utils.run_bass_kernel_spmd(
            nc, in_maps, core_ids=list(range(N_CORES)), trace=TRACE)
    except ModuleNotFoundError:
        # no axon NTFF profile hook in this environment
        res = bass_utils.run_bass_kernel_spmd(
            nc, in_maps, core_ids=list(range(N_CORES)), trace=False)
    _LAST_RESULTS["res"] = res

    # [core, img, ob, 128, 56, 64] bf16 -> [16, 512, 56, 56] f32
    out = np.stack([np.asarray(r["out"])[..., :W] for r in res.results],
                   axis=0)
    return np.ascontiguousarray(out.astype(np.float32)).reshape(
        16, COUT, H, W)
